# revision 12
# baseline (speedup 1.0000x reference)
"""3x3 conv (im2col formulation) as 9 shifted matmuls on TRN2, data-parallel over batch.

Full inputs: x [32, 128, 56, 56] f32, w [1152, 256] f32 (row = c*9 + kh*3 + kw).
Full output: [32, 256, 56, 56] f32.

Each of the 8 cores processes 4 batch images. Per core:
  - x image is DMA'd contiguously into a [128(c), 56, 56] staging tile
    (full-rate 12.5KB/partition descriptors), then copied on-chip into a
    zero-bordered [128, 58, 58] padded tile (f32r).
  - w is DMA'd once as [128(c), 9, 256] (f32r).
  - Per (image, out-channel half, 8-row band): 9 shifted matmuls accumulate
    w_tap.T @ x_shifted into a [128(o), 8, 56] PSUM bank (f32r = full PE
    rate, ~1e-4 rel err); DVE copies the band to a [128, 56, 56] SBUF image;
    each band streams to DRAM immediately (1792B/partition contiguous) on
    the scalar-engine HWDGE queue.
  - Dummy bf16 warmup matmuls run during the DMA lead-in to lift the PE HAM
    clock gate before the real stream starts.
"""

import numpy as np

import concourse.bass as bass  # noqa: F401  (registers AP types)
import concourse.mybir as mybir
import concourse.tile as tile
from concourse import bacc, bass_utils

B, C, H, W = 32, 128, 56, 56
COUT = 256
NCORES = 8
BPC = B // NCORES  # images per core
HP, WP = H + 2, W + 2
HROWS = 8  # output rows per PSUM band
HT = H // HROWS  # bands per image
F32 = mybir.dt.float32
F32R = mybir.dt.float32r
BF16 = mybir.dt.bfloat16

_cached_nc = None


def _build():
    nc = bacc.Bacc(None, target_bir_lowering=False)
    x = nc.dram_tensor("x", [BPC, C, H, W], F32, kind="ExternalInput")
    w = nc.dram_tensor("w", [C * 9, COUT], F32, kind="ExternalInput")
    out = nc.dram_tensor("out", [BPC, COUT, H, W], F32, kind="ExternalOutput")
    wr = w.rearrange("(c k) o -> c k o", k=9)

    with tile.TileContext(nc) as tc:
        with (
            tc.tile_pool(name="wpool", bufs=1) as wpool,
            tc.tile_pool(name="spool", bufs=2) as spool,
            tc.tile_pool(name="xpool", bufs=2) as xpool,
            tc.tile_pool(name="opool", bufs=2) as opool,
            tc.tile_pool(name="pspool", bufs=8, space="PSUM") as pspool,
        ):
            # PE warmup: tiny matmuls with no data deps keep the PE busy
            # during the input DMA so HAM reaches K=8/8 before the real work.
            NWARM = 300
            warm = wpool.tile([C, 16], BF16)
            nc.vector.memset(warm[:], 0.0)
            wpsum = pspool.tile([16, 16], F32, tag="pt", name="warm_psum")
            for i in range(NWARM):
                nc.tensor.matmul(wpsum[:], warm[:, :16], warm[:, :16],
                                 start=(i == 0), stop=(i == NWARM - 1))

            # Image 0 is the critical path: its load goes first on the sync
            # ring while w rides the scalar ring in parallel.
            wbuf = wpool.tile([C, 9, COUT], F32R)
            for b in range(BPC):
                xs = spool.tile([C, H, W], F32R, tag="xs", name=f"xs{b}")
                nc.sync.dma_start(xs[:], x[b].bitcast(F32R))
                if b == 0:
                    nc.scalar.dma_start(wbuf[:], wr[:].bitcast(F32R))

                xp = xpool.tile([C, HP, WP], F32R, tag="xp", name=f"xp{b}")
                nc.vector.memset(xp[:, 0, :].bitcast(F32), 0.0)
                nc.vector.memset(xp[:, HP - 1, :].bitcast(F32), 0.0)
                nc.vector.memset(xp[:, :, 0].bitcast(F32), 0.0)
                nc.vector.memset(xp[:, :, WP - 1].bitcast(F32), 0.0)
                nc.vector.tensor_copy(out=xp[:, 1 : H + 1, 1 : W + 1], in_=xs[:])

                for oc in range(COUT // 128):
                    oimg = opool.tile([128, H, W], F32, tag="oimg", name=f"oimg{b}_{oc}")
                    for ht in range(HT):
                        pt = pspool.tile(
                            [128, HROWS, W], F32, tag="pt", name=f"pt{b}_{oc}_{ht}"
                        )
                        for dh in (-1, 0, 1):
                            for dw in (-1, 0, 1):
                                kk = (dh + 1) * 3 + (dw + 1)
                                h0 = ht * HROWS + dh + 1
                                rhs = xp[:, h0 : h0 + HROWS, dw + 1 : dw + 1 + W]
                                lhsT = wbuf[:, kk, oc * 128 : (oc + 1) * 128]
                                nc.tensor.matmul(
                                    pt[:], lhsT, rhs, start=(kk == 0), stop=(kk == 8)
                                )
                        nc.vector.tensor_copy(
                            out=oimg[:, ht * HROWS : (ht + 1) * HROWS, :], in_=pt[:]
                        )
                        nc.scalar.dma_start(
                            out[b, oc * 128 : (oc + 1) * 128,
                                ht * HROWS : (ht + 1) * HROWS, :],
                            oimg[:, ht * HROWS : (ht + 1) * HROWS, :],
                        )
    nc.compile()
    return nc


def _get_nc():
    global _cached_nc
    if _cached_nc is None:
        _cached_nc = _build()
    return _cached_nc


def run(x, w, trace=False, **spmd_kwargs):
    nc = _get_nc()
    x = np.ascontiguousarray(x, dtype=np.float32)
    w = np.ascontiguousarray(w, dtype=np.float32)
    in_maps = [
        {"x": x[i * BPC : (i + 1) * BPC], "w": w} for i in range(NCORES)
    ]
    res = bass_utils.run_bass_kernel_spmd(
        nc, in_maps, core_ids=list(range(NCORES)), trace=trace, **spmd_kwargs
    )
    full = np.concatenate([r["out"] for r in res.results], axis=0)
    return full, res


def kernel(x, w):
    return run(x, w)[0]


# revision 16
# speedup vs baseline: 1.0510x; 1.0510x over previous
"""3x3 conv (im2col formulation) as 9 shifted matmuls on TRN2, data-parallel over batch.

Full inputs: x [32, 128, 56, 56] f32, w [1152, 256] f32 (row = c*9 + kh*3 + kw).
Full output: [32, 256, 56, 56] f32.

Each of the 8 cores processes 4 batch images. Per core:
  - x image is DMA'd contiguously into a [128(c), 56, 56] staging tile
    (full-rate 12.5KB/partition descriptors), then copied on-chip into a
    zero-bordered [128, 58, 58] padded tile (f32r).
  - w is DMA'd once as [128(c), 9, 256] (f32r).
  - Per (image, out-channel half, 8-row band): 9 shifted matmuls accumulate
    w_tap.T @ x_shifted into a [128(o), 8, 56] PSUM bank (f32r = full PE
    rate, ~1e-4 rel err); DVE copies the band to a [128, 56, 56] SBUF image;
    each band streams to DRAM immediately (1792B/partition contiguous) on
    the scalar-engine HWDGE queue.
  - Dummy bf16 warmup matmuls run during the DMA lead-in to lift the PE HAM
    clock gate before the real stream starts.
"""

import numpy as np

import concourse.bass as bass  # noqa: F401  (registers AP types)
import concourse.mybir as mybir
import concourse.tile as tile
from concourse import bacc, bass_utils

B, C, H, W = 32, 128, 56, 56
COUT = 256
NCORES = 8
BPC = B // NCORES  # images per core
HP, WP = H + 2, W + 2
HROWS = 8  # output rows per PSUM band
HT = H // HROWS  # bands per image
F32 = mybir.dt.float32
F32R = mybir.dt.float32r
BF16 = mybir.dt.bfloat16

_cached_nc = None


def _build():
    nc = bacc.Bacc(None, target_bir_lowering=False)
    x = nc.dram_tensor("x", [BPC, C, H, W], F32, kind="ExternalInput")
    # host pre-arranges w as [oc_half, c, tap, 128] so each half DMAs with
    # fully contiguous per-partition chunks
    w = nc.dram_tensor("w", [2, C, 9, 128], F32, kind="ExternalInput")
    out = nc.dram_tensor("out", [BPC, COUT, H, W], F32, kind="ExternalOutput")

    with tile.TileContext(nc) as tc:
        with (
            tc.tile_pool(name="wpool", bufs=1) as wpool,
            tc.tile_pool(name="spool", bufs=2) as spool,
            tc.tile_pool(name="xpool", bufs=2) as xpool,
            tc.tile_pool(name="opool", bufs=2) as opool,
            tc.tile_pool(name="pspool", bufs=8, space="PSUM") as pspool,
        ):
            # PE warmup: tiny matmuls with no data deps keep the PE busy
            # during the input DMA so HAM reaches K=8/8 before the real work.
            NWARM = 220
            warm = wpool.tile([C, 16], BF16)
            nc.vector.memset(warm[:], 0.0)
            wpsum = pspool.tile([16, 16], F32, tag="pt", name="warm_psum")
            for i in range(NWARM):
                nc.tensor.matmul(wpsum[:], warm[:, :16], warm[:, :16],
                                 start=(i == 0), stop=(i == NWARM - 1))

            # Input bandwidth is shared (~330GB/s), so sequence the sync-ring
            # DMAs to put the minimum bytes ahead of the first matmul:
            # xs0-lower, w-oc0, xs0-upper, w-oc1, then the remaining images.
            # The image-0 pad copy is split so bands 0-2 start after the
            # lower half lands.
            HSPL = 28
            wbuf = wpool.tile([C, 2, 9, 128], F32R)
            xs0 = spool.tile([C, H, W], F32R, tag="xs", name="xs0")
            nc.sync.dma_start(xs0[:, :HSPL, :], x[0, :, :HSPL, :].bitcast(F32R))
            nc.sync.dma_start(wbuf[:, 0], w[0].bitcast(F32R))
            nc.sync.dma_start(xs0[:, HSPL:, :], x[0, :, HSPL:, :].bitcast(F32R))
            nc.sync.dma_start(wbuf[:, 1], w[1].bitcast(F32R))

            for b in range(BPC):
                if b == 0:
                    xs = xs0
                else:
                    xs = spool.tile([C, H, W], F32R, tag="xs", name=f"xs{b}")
                    nc.sync.dma_start(xs[:], x[b].bitcast(F32R))

                xp = xpool.tile([C, HP, WP], F32R, tag="xp", name=f"xp{b}")
                nc.vector.memset(xp[:, 0, :].bitcast(F32), 0.0)
                nc.vector.memset(xp[:, HP - 1, :].bitcast(F32), 0.0)
                nc.vector.memset(xp[:, :, 0].bitcast(F32), 0.0)
                nc.vector.memset(xp[:, :, WP - 1].bitcast(F32), 0.0)
                if b == 0:
                    nc.vector.tensor_copy(
                        out=xp[:, 1 : HSPL + 1, 1 : W + 1], in_=xs[:, :HSPL, :]
                    )
                    nc.vector.tensor_copy(
                        out=xp[:, HSPL + 1 : H + 1, 1 : W + 1], in_=xs[:, HSPL:, :]
                    )
                else:
                    nc.vector.tensor_copy(out=xp[:, 1 : H + 1, 1 : W + 1], in_=xs[:])

                for oc in range(COUT // 128):
                    oimg = opool.tile([128, H, W], F32, tag="oimg", name=f"oimg{b}_{oc}")
                    for ht in range(HT):
                        pt = pspool.tile(
                            [128, HROWS, W], F32, tag="pt", name=f"pt{b}_{oc}_{ht}"
                        )
                        for dh in (-1, 0, 1):
                            for dw in (-1, 0, 1):
                                kk = (dh + 1) * 3 + (dw + 1)
                                h0 = ht * HROWS + dh + 1
                                rhs = xp[:, h0 : h0 + HROWS, dw + 1 : dw + 1 + W]
                                lhsT = wbuf[:, oc, kk, :]
                                nc.tensor.matmul(
                                    pt[:], lhsT, rhs, start=(kk == 0), stop=(kk == 8)
                                )
                        nc.vector.tensor_copy(
                            out=oimg[:, ht * HROWS : (ht + 1) * HROWS, :], in_=pt[:]
                        )
                        nc.scalar.dma_start(
                            out[b, oc * 128 : (oc + 1) * 128,
                                ht * HROWS : (ht + 1) * HROWS, :],
                            oimg[:, ht * HROWS : (ht + 1) * HROWS, :],
                        )
    nc.compile()
    return nc


def _get_nc():
    global _cached_nc
    if _cached_nc is None:
        _cached_nc = _build()
    return _cached_nc


def run(x, w, trace=False, **spmd_kwargs):
    nc = _get_nc()
    x = np.ascontiguousarray(x, dtype=np.float32)
    w = np.asarray(w, dtype=np.float32)
    # [c*9, 256] -> [oc_half, c, tap, 128]
    w2 = np.ascontiguousarray(
        w.reshape(C, 9, 2, 128).transpose(2, 0, 1, 3)
    )
    in_maps = [
        {"x": x[i * BPC : (i + 1) * BPC], "w": w2} for i in range(NCORES)
    ]
    res = bass_utils.run_bass_kernel_spmd(
        nc, in_maps, core_ids=list(range(NCORES)), trace=trace, **spmd_kwargs
    )
    full = np.concatenate([r["out"] for r in res.results], axis=0)
    return full, res


def kernel(x, w):
    return run(x, w)[0]
